# revision 4
# baseline (speedup 1.0000x reference)
"""Trainium2 Bass kernel for nn_Mesh2_14267881357853 (gnn_message_passing).

Computation (reference):
    out3 = concat(out1, out2) @ W_comb.T + b_comb              [N, 512]
    agg  = (out2 + sum_j out2[neighbour[:, j]]) * 0.25         [N, 256]
    out4 = agg @ W_agg.T + b_agg                               [N, 512]

Strategy: data-parallel over nodes, 8 cores x 25088 rows (25000 real + pad).
Weights replicated; full out2 (bf16) replicated per core for the neighbour
row gather via SWDGE indirect DMA (batched: one gather instr covers
TB tiles x 3 neighbours to amortize the ~1us Q7 descriptor-emission cost).
Activations are pre-transposed host-side into feature-major [feat, node]
tiles so they feed the PE directly as lhsT; gathered neighbour rows are
node-major and get PE-transposed, accumulating all 3 neighbours into PSUM.
Compute in bf16 (f32 PSUM accumulate), outputs f32.
"""

import numpy as np
import ml_dtypes
from contextlib import ExitStack

import concourse.bass as bass
import concourse.tile as tile
from concourse import bacc, mybir
from concourse.bass_utils import run_bass_kernel_spmd
from concourse.masks import make_identity

BF16 = ml_dtypes.bfloat16
P = 128
NCORES = 8
N_FULL = 200000
RPC = N_FULL // NCORES          # 25000 real rows per core
TB = 7                          # node-tiles per batch
NB = 28                         # batches  -> 196 tiles = 25088 padded rows
NP_PAD = NB * TB * P            # 25088
D_IN = 256
D_OUT = 512


def build_program(nb=NB, tb=TB, n_full=N_FULL, n_cores=NCORES):
    """Build the SPMD Bass program (same program for all cores)."""
    dt = mybir.dt
    npad = nb * tb * P
    nc = bacc.Bacc(
        "TRN2",
        target_bir_lowering=False,
        debug=False,
        enable_asserts=True,
        num_devices=n_cores,
    )
    x1d = nc.dram_tensor("x1t", [nb, P, tb * 2, P], dt.bfloat16, kind="ExternalInput").ap()
    a2d = nc.dram_tensor("a2t", [nb, P, tb * 2, P], dt.bfloat16, kind="ExternalInput").ap()
    idxd = nc.dram_tensor("idx", [P, nb * tb * 3], dt.int32, kind="ExternalInput").ap()
    o2fd = nc.dram_tensor("o2f", [n_full, D_IN], dt.bfloat16, kind="ExternalInput").ap()
    wctd = nc.dram_tensor("wct", [4, P, D_OUT], dt.bfloat16, kind="ExternalInput").ap()
    wagd = nc.dram_tensor("wagt", [2, P, D_OUT], dt.bfloat16, kind="ExternalInput").ap()
    bcd = nc.dram_tensor("bc", [P, D_OUT], dt.float32, kind="ExternalInput").ap()
    bad = nc.dram_tensor("ba", [P, D_OUT], dt.float32, kind="ExternalInput").ap()
    o3d = nc.dram_tensor("o3", [npad, D_OUT], dt.float32, kind="ExternalOutput").ap()
    o4d = nc.dram_tensor("o4", [npad, D_OUT], dt.float32, kind="ExternalOutput").ap()

    with tile.TileContext(nc) as tc, ExitStack() as ctx:
        const = ctx.enter_context(tc.tile_pool(name="const", bufs=1))
        loadp = ctx.enter_context(tc.tile_pool(name="loads", bufs=2))
        work = ctx.enter_context(tc.tile_pool(name="work", bufs=3))
        outp = ctx.enter_context(tc.tile_pool(name="outs", bufs=3))
        pst = ctx.enter_context(tc.tile_pool(name="pst", bufs=2, space="PSUM"))
        psm = ctx.enter_context(tc.tile_pool(name="psm", bufs=2, space="PSUM"))

        ident = const.tile([P, P], dt.float32)
        make_identity(nc, ident[:])
        idx_sb = const.tile([P, nb * tb * 3], dt.int32)
        nc.sync.dma_start(idx_sb[:], idxd[:])
        wct_sb = const.tile([P, 4, D_OUT], dt.bfloat16)
        for c in range(4):
            nc.sync.dma_start(wct_sb[:, c, :], wctd[c])
        wag_sb = const.tile([P, 2, D_OUT], dt.bfloat16)
        for c in range(2):
            nc.sync.dma_start(wag_sb[:, c, :], wagd[c])
        bc_sb = const.tile([P, D_OUT], dt.float32)
        nc.sync.dma_start(bc_sb[:], bcd[:])
        ba_sb = const.tile([P, D_OUT], dt.float32)
        nc.sync.dma_start(ba_sb[:], bad[:])

        for b in range(nb):
            x1 = loadp.tile([P, tb * 2, P], dt.bfloat16, tag="x1")
            nc.sync.dma_start(x1[:], x1d[b])
            a2 = loadp.tile([P, tb * 2, P], dt.bfloat16, tag="a2")
            nc.sync.dma_start(a2[:], a2d[b])
            # one batched gather: tb tiles x 3 neighbours x 256 feats
            g = loadp.tile([P, tb * 3, D_IN], dt.bfloat16, tag="g")
            nc.gpsimd.indirect_dma_start(
                out=g[:],
                out_offset=None,
                in_=o2fd[:],
                in_offset=bass.IndirectOffsetOnAxis(
                    ap=idx_sb[:, b * tb * 3:(b + 1) * tb * 3], axis=0
                ),
            )
            for t in range(tb):
                # sum the 3 gathered neighbour tiles (node-major) in f32
                gsum = work.tile([P, D_IN], dt.float32, tag="gsum")
                nc.vector.tensor_tensor(
                    out=gsum[:], in0=g[:, t * 3 + 0, :], in1=g[:, t * 3 + 1, :],
                    op=mybir.AluOpType.add,
                )
                nc.vector.tensor_tensor(
                    out=gsum[:], in0=gsum[:], in1=g[:, t * 3 + 2, :],
                    op=mybir.AluOpType.add,
                )
                # PE-transpose the sum to feature-major (f32 -> f32 PSUM)
                pt = pst.tile([P, 2, P], dt.float32, tag="pt")
                for c in range(2):
                    nc.tensor.transpose(
                        out=pt[:, c, :],
                        in_=gsum[:, c * P:(c + 1) * P],
                        identity=ident[:],
                    )
                # evict + add self rows (already feature-major) -> bf16 aggT
                aggt = work.tile([P, 2, P], dt.bfloat16, tag="aggt")
                for c in range(2):
                    nc.vector.tensor_tensor(
                        out=aggt[:, c, :], in0=pt[:, c, :], in1=a2[:, t * 2 + c, :],
                        op=mybir.AluOpType.add,
                    )
                # comb: concat(out1,out2) @ W_comb.T  (4 K-chunks of 128)
                p3 = psm.tile([P, D_OUT], dt.float32, tag="p3")
                for c in range(4):
                    lhsT = x1[:, t * 2 + c, :] if c < 2 else a2[:, t * 2 + c - 2, :]
                    nc.tensor.matmul(
                        out=p3[:], lhsT=lhsT, rhs=wct_sb[:, c, :],
                        start=(c == 0), stop=(c == 3),
                    )
                # agg matmul (scale 0.25 folded into W_agg host-side)
                p4 = psm.tile([P, D_OUT], dt.float32, tag="p4")
                for c in range(2):
                    nc.tensor.matmul(
                        out=p4[:], lhsT=aggt[:, c, :], rhs=wag_sb[:, c, :],
                        start=(c == 0), stop=(c == 1),
                    )
                r0 = (b * tb + t) * P
                o3sb = outp.tile([P, D_OUT], dt.float32, tag="o3sb")
                nc.vector.tensor_tensor(
                    out=o3sb[:], in0=p3[:], in1=bc_sb[:], op=mybir.AluOpType.add
                )
                nc.scalar.dma_start(o3d[r0:r0 + P, :], o3sb[:])
                o4sb = outp.tile([P, D_OUT], dt.float32, tag="o4sb")
                nc.vector.tensor_tensor(
                    out=o4sb[:], in0=p4[:], in1=ba_sb[:], op=mybir.AluOpType.add
                )
                nc.scalar.dma_start(o4d[r0:r0 + P, :], o4sb[:])

    nc.compile()
    return nc


def _pack_T(rows, nb, tb):
    """[rows, 256] f32 -> [nb, P, tb, 2, P] bf16 feature-major tiles."""
    npad = nb * tb * P
    pad = np.zeros((npad, D_IN), BF16)
    pad[: rows.shape[0]] = rows.astype(BF16)
    r = pad.reshape(nb, tb, P, 2, P)               # [b, t, node, c, feat]
    return np.ascontiguousarray(r.transpose(0, 4, 1, 3, 2)).reshape(nb, P, tb * 2, P)  # [b, feat, t*2+c, node]


def _pack_idx(nbr, nb, tb):
    """[rows, 3] int32 -> [P, nb*tb*3] partition-major index layout."""
    npad = nb * tb * P
    pad = np.zeros((npad, 3), np.int32)
    pad[: nbr.shape[0]] = nbr
    r = pad.reshape(nb, tb, P, 3)                  # [b, t, node, j]
    return np.ascontiguousarray(r.transpose(2, 0, 1, 3).reshape(P, nb * tb * 3))


def prep_in_maps(out1, out2, neighbour, W_comb, b_comb, W_agg, b_agg,
                 nb=NB, tb=TB, n_cores=NCORES):
    out1 = np.asarray(out1, dtype=np.float32)
    out2 = np.asarray(out2, dtype=np.float32)
    nbr32 = np.asarray(neighbour).astype(np.int32)
    o2f = np.ascontiguousarray(np.asarray(out2).astype(BF16))
    wct = np.ascontiguousarray(np.asarray(W_comb, dtype=np.float32).T.astype(BF16)).reshape(4, P, D_OUT)
    wag = np.ascontiguousarray((0.25 * np.asarray(W_agg, dtype=np.float32)).T.astype(BF16)).reshape(2, P, D_OUT)
    bc = np.ascontiguousarray(np.tile(np.asarray(b_comb, dtype=np.float32)[None, :], (P, 1)))
    ba = np.ascontiguousarray(np.tile(np.asarray(b_agg, dtype=np.float32)[None, :], (P, 1)))
    rpc = out1.shape[0] // n_cores
    in_maps = []
    for i in range(n_cores):
        sl = slice(i * rpc, (i + 1) * rpc)
        in_maps.append(dict(
            x1t=_pack_T(out1[sl], nb, tb),
            a2t=_pack_T(out2[sl], nb, tb),
            idx=_pack_idx(nbr32[sl], nb, tb),
            o2f=o2f, wct=wct, wagt=wag, bc=bc, ba=ba,
        ))
    return in_maps


_NC_CACHE = {}


def _get_program():
    key = (NB, TB, N_FULL)
    if key not in _NC_CACHE:
        _NC_CACHE[key] = build_program()
    return _NC_CACHE[key]


def kernel(out1, out2, neighbour, W_comb, b_comb, W_agg, b_agg, _trace=False, **kw):
    nc = _get_program()
    in_maps = prep_in_maps(out1, out2, neighbour, W_comb, b_comb, W_agg, b_agg)
    res = run_bass_kernel_spmd(nc, in_maps, list(range(NCORES)), trace=_trace, **kw)
    out3 = np.concatenate([res.results[i]["o3"][:RPC] for i in range(NCORES)], axis=0)
    out4 = np.concatenate([res.results[i]["o4"][:RPC] for i in range(NCORES)], axis=0)
    if _trace:
        return (out3, out4), res
    return (out3, out4)


# revision 5
# speedup vs baseline: 1.0433x; 1.0433x over previous
"""Trainium2 Bass kernel for nn_Mesh2_14267881357853 (gnn_message_passing).

Computation (reference):
    out3 = concat(out1, out2) @ W_comb.T + b_comb              [N, 512]
    agg  = (out2 + sum_j out2[neighbour[:, j]]) * 0.25         [N, 256]
    out4 = agg @ W_agg.T + b_agg                               [N, 512]

Strategy: data-parallel over nodes, 8 cores x 25088 rows (25000 real + pad).
Weights replicated; full out2 (bf16) replicated per core for the neighbour
row gather via SWDGE indirect DMA (batched: one gather instr covers
TB tiles x 3 neighbours to amortize the ~1us Q7 descriptor-emission cost).
Activations are pre-transposed host-side into feature-major [feat, node]
tiles so they feed the PE directly as lhsT; gathered neighbour rows are
node-major and get PE-transposed, accumulating all 3 neighbours into PSUM.
Compute in bf16 (f32 PSUM accumulate), outputs f32.
"""

import numpy as np
import ml_dtypes
from contextlib import ExitStack

import concourse.bass as bass
import concourse.tile as tile
from concourse import bacc, mybir
from concourse.bass_utils import run_bass_kernel_spmd
from concourse.masks import make_identity

BF16 = ml_dtypes.bfloat16
P = 128
NCORES = 8
N_FULL = 200000
RPC = N_FULL // NCORES          # 25000 real rows per core
TB = 7                          # node-tiles per batch
NB = 28                         # batches  -> 196 tiles = 25088 padded rows
NP_PAD = NB * TB * P            # 25088
D_IN = 256
D_OUT = 512


def build_program(nb=NB, tb=TB, n_full=N_FULL, n_cores=NCORES):
    """Build the SPMD Bass program (same program for all cores)."""
    dt = mybir.dt
    npad = nb * tb * P
    nc = bacc.Bacc(
        "TRN2",
        target_bir_lowering=False,
        debug=False,
        enable_asserts=True,
        num_devices=n_cores,
    )
    x1d = nc.dram_tensor("x1t", [nb, P, tb * 2, P], dt.bfloat16, kind="ExternalInput").ap()
    a2d = nc.dram_tensor("a2t", [nb, P, tb * 2, P], dt.bfloat16, kind="ExternalInput").ap()
    idxd = nc.dram_tensor("idx", [P, nb * tb * 3], dt.int32, kind="ExternalInput").ap()
    o2fd = nc.dram_tensor("o2f", [n_full, D_IN], dt.bfloat16, kind="ExternalInput").ap()
    wctd = nc.dram_tensor("wct", [4, P, D_OUT], dt.bfloat16, kind="ExternalInput").ap()
    wagd = nc.dram_tensor("wagt", [2, P, D_OUT], dt.bfloat16, kind="ExternalInput").ap()
    bcd = nc.dram_tensor("bc", [P, D_OUT], dt.float32, kind="ExternalInput").ap()
    bad = nc.dram_tensor("ba", [P, D_OUT], dt.float32, kind="ExternalInput").ap()
    o3d = nc.dram_tensor("o3", [npad, D_OUT], dt.float32, kind="ExternalOutput").ap()
    o4d = nc.dram_tensor("o4", [npad, D_OUT], dt.float32, kind="ExternalOutput").ap()

    with tile.TileContext(nc) as tc, ExitStack() as ctx:
        const = ctx.enter_context(tc.tile_pool(name="const", bufs=1))
        loadp = ctx.enter_context(tc.tile_pool(name="loads", bufs=2))
        work = ctx.enter_context(tc.tile_pool(name="work", bufs=3))
        outp = ctx.enter_context(tc.tile_pool(name="outs", bufs=3))
        pst = ctx.enter_context(tc.tile_pool(name="pst", bufs=2, space="PSUM"))
        psm = ctx.enter_context(tc.tile_pool(name="psm", bufs=2, space="PSUM"))

        ident = const.tile([P, P], dt.float32)
        make_identity(nc, ident[:])
        idx_sb = const.tile([P, nb * tb * 3], dt.int32)
        nc.sync.dma_start(idx_sb[:], idxd[:])
        wct_sb = const.tile([P, 4, D_OUT], dt.bfloat16)
        for c in range(4):
            nc.sync.dma_start(wct_sb[:, c, :], wctd[c])
        wag_sb = const.tile([P, 2, D_OUT], dt.bfloat16)
        for c in range(2):
            nc.sync.dma_start(wag_sb[:, c, :], wagd[c])
        bc_sb = const.tile([P, D_OUT], dt.float32)
        nc.sync.dma_start(bc_sb[:], bcd[:])
        ba_sb = const.tile([P, D_OUT], dt.float32)
        nc.sync.dma_start(ba_sb[:], bad[:])

        for b in range(nb):
            x1 = loadp.tile([P, tb * 2, P], dt.bfloat16, tag="x1")
            nc.sync.dma_start(x1[:], x1d[b])
            a2 = loadp.tile([P, tb * 2, P], dt.bfloat16, tag="a2")
            nc.sync.dma_start(a2[:], a2d[b])
            # gathers: HW ucode supports one index per partition per instr
            g = loadp.tile([P, tb * 3, D_IN], dt.bfloat16, tag="g")
            for k in range(tb * 3):
                ki = b * tb * 3 + k
                nc.gpsimd.indirect_dma_start(
                    out=g[:, k, :],
                    out_offset=None,
                    in_=o2fd[:],
                    in_offset=bass.IndirectOffsetOnAxis(
                        ap=idx_sb[:, ki:ki + 1], axis=0
                    ),
                )
            for t in range(tb):
                # sum the 3 gathered neighbour tiles (node-major) in f32
                gsum = work.tile([P, D_IN], dt.float32, tag="gsum")
                nc.vector.tensor_tensor(
                    out=gsum[:], in0=g[:, t * 3 + 0, :], in1=g[:, t * 3 + 1, :],
                    op=mybir.AluOpType.add,
                )
                nc.vector.tensor_tensor(
                    out=gsum[:], in0=gsum[:], in1=g[:, t * 3 + 2, :],
                    op=mybir.AluOpType.add,
                )
                # PE-transpose the sum to feature-major (f32 -> f32 PSUM)
                pt = pst.tile([P, 2, P], dt.float32, tag="pt")
                for c in range(2):
                    nc.tensor.transpose(
                        out=pt[:, c, :],
                        in_=gsum[:, c * P:(c + 1) * P],
                        identity=ident[:],
                    )
                # evict + add self rows (already feature-major) -> bf16 aggT
                aggt = work.tile([P, 2, P], dt.bfloat16, tag="aggt")
                for c in range(2):
                    nc.vector.tensor_tensor(
                        out=aggt[:, c, :], in0=pt[:, c, :], in1=a2[:, t * 2 + c, :],
                        op=mybir.AluOpType.add,
                    )
                # comb: concat(out1,out2) @ W_comb.T  (4 K-chunks of 128)
                p3 = psm.tile([P, D_OUT], dt.float32, tag="p3")
                for c in range(4):
                    lhsT = x1[:, t * 2 + c, :] if c < 2 else a2[:, t * 2 + c - 2, :]
                    nc.tensor.matmul(
                        out=p3[:], lhsT=lhsT, rhs=wct_sb[:, c, :],
                        start=(c == 0), stop=(c == 3),
                    )
                # agg matmul (scale 0.25 folded into W_agg host-side)
                p4 = psm.tile([P, D_OUT], dt.float32, tag="p4")
                for c in range(2):
                    nc.tensor.matmul(
                        out=p4[:], lhsT=aggt[:, c, :], rhs=wag_sb[:, c, :],
                        start=(c == 0), stop=(c == 1),
                    )
                r0 = (b * tb + t) * P
                o3sb = outp.tile([P, D_OUT], dt.float32, tag="o3sb")
                nc.vector.tensor_tensor(
                    out=o3sb[:], in0=p3[:], in1=bc_sb[:], op=mybir.AluOpType.add
                )
                nc.scalar.dma_start(o3d[r0:r0 + P, :], o3sb[:])
                o4sb = outp.tile([P, D_OUT], dt.float32, tag="o4sb")
                nc.vector.tensor_tensor(
                    out=o4sb[:], in0=p4[:], in1=ba_sb[:], op=mybir.AluOpType.add
                )
                nc.scalar.dma_start(o4d[r0:r0 + P, :], o4sb[:])

    nc.compile()
    return nc


def _pack_T(rows, nb, tb):
    """[rows, 256] f32 -> [nb, P, tb, 2, P] bf16 feature-major tiles."""
    npad = nb * tb * P
    pad = np.zeros((npad, D_IN), BF16)
    pad[: rows.shape[0]] = rows.astype(BF16)
    r = pad.reshape(nb, tb, P, 2, P)               # [b, t, node, c, feat]
    return np.ascontiguousarray(r.transpose(0, 4, 1, 3, 2)).reshape(nb, P, tb * 2, P)  # [b, feat, t*2+c, node]


def _pack_idx(nbr, nb, tb):
    """[rows, 3] int32 -> [P, nb*tb*3] partition-major index layout."""
    npad = nb * tb * P
    pad = np.zeros((npad, 3), np.int32)
    pad[: nbr.shape[0]] = nbr
    r = pad.reshape(nb, tb, P, 3)                  # [b, t, node, j]
    return np.ascontiguousarray(r.transpose(2, 0, 1, 3).reshape(P, nb * tb * 3))


def prep_in_maps(out1, out2, neighbour, W_comb, b_comb, W_agg, b_agg,
                 nb=NB, tb=TB, n_cores=NCORES):
    out1 = np.asarray(out1, dtype=np.float32)
    out2 = np.asarray(out2, dtype=np.float32)
    nbr32 = np.asarray(neighbour).astype(np.int32)
    o2f = np.ascontiguousarray(np.asarray(out2).astype(BF16))
    wct = np.ascontiguousarray(np.asarray(W_comb, dtype=np.float32).T.astype(BF16)).reshape(4, P, D_OUT)
    wag = np.ascontiguousarray((0.25 * np.asarray(W_agg, dtype=np.float32)).T.astype(BF16)).reshape(2, P, D_OUT)
    bc = np.ascontiguousarray(np.tile(np.asarray(b_comb, dtype=np.float32)[None, :], (P, 1)))
    ba = np.ascontiguousarray(np.tile(np.asarray(b_agg, dtype=np.float32)[None, :], (P, 1)))
    rpc = out1.shape[0] // n_cores
    in_maps = []
    for i in range(n_cores):
        sl = slice(i * rpc, (i + 1) * rpc)
        in_maps.append(dict(
            x1t=_pack_T(out1[sl], nb, tb),
            a2t=_pack_T(out2[sl], nb, tb),
            idx=_pack_idx(nbr32[sl], nb, tb),
            o2f=o2f, wct=wct, wagt=wag, bc=bc, ba=ba,
        ))
    return in_maps


_NC_CACHE = {}


def _get_program():
    key = (NB, TB, N_FULL)
    if key not in _NC_CACHE:
        _NC_CACHE[key] = build_program()
    return _NC_CACHE[key]


def kernel(out1, out2, neighbour, W_comb, b_comb, W_agg, b_agg, _trace=False, **kw):
    nc = _get_program()
    in_maps = prep_in_maps(out1, out2, neighbour, W_comb, b_comb, W_agg, b_agg)
    res = run_bass_kernel_spmd(nc, in_maps, list(range(NCORES)), trace=_trace, **kw)
    out3 = np.concatenate([res.results[i]["o3"][:RPC] for i in range(NCORES)], axis=0)
    out4 = np.concatenate([res.results[i]["o4"][:RPC] for i in range(NCORES)], axis=0)
    if _trace:
        return (out3, out4), res
    return (out3, out4)


# revision 31
# speedup vs baseline: 1.6286x; 1.5610x over previous
"""Trainium2 Bass kernel for nn_Mesh2_14267881357853 (gnn_message_passing).

Computation (reference):
    out3 = concat(out1, out2) @ W_comb.T + b_comb              [N, 512]
    agg  = (out2 + sum_j out2[neighbour[:, j]]) * 0.25         [N, 256]
    out4 = agg @ W_agg.T + b_agg                               [N, 512]

Strategy: data-parallel over nodes, 8 cores x 25088 rows (25000 real + pad).
Weights replicated; full out2 (bf16) replicated per core for the neighbour
row gather via SWDGE indirect DMA ([P,1] indices -- the HW ucode supports one
index per partition per instruction; multi-index offset APs gather garbage).
The 3 neighbour gathers per node-tile accumulate directly in SDMA
(compute_op=add, bf16->f32 cast), so the vector engine never sees them.
Activations are pre-transposed host-side into feature-major [feat, node]
tiles that feed the PE directly as lhsT; the gathered neighbour-sum is
node-major and gets PE-transposed, then the self rows are added during the
DVE eviction to bf16. Biases ride a DVE eviction add (out3, 3/4 of out4
tiles) or a K=1 ones x bias_row PE matmul + ACT-copy eviction (1/4 of out4
tiles) to balance engines. Compute bf16 (f32 PSUM accumulate); outputs are
stored bf16 and upcast to f32 on the host (halves store traffic; adds
~0.2% relative error on top of the bf16-compute error, total ~0.4%).
Cost-model (CoreSim no_exec) estimate: ~319 us for the 8-core NEFF;
PE/DVE/Pool all ~92-96% occupied.
"""

import numpy as np
import ml_dtypes
from contextlib import ExitStack

import concourse.bass as bass
import concourse.tile as tile
from concourse import bacc, mybir
from concourse.bass_utils import run_bass_kernel_spmd
from concourse.masks import make_identity

BF16 = ml_dtypes.bfloat16
P = 128
NCORES = 8
N_FULL = 200000
RPC = N_FULL // NCORES          # 25000 real rows per core
TB = 7                          # node-tiles per batch
NB = 28                         # batches  -> 196 tiles = 25088 padded rows
NP_PAD = NB * TB * P            # 25088
D_IN = 256
D_OUT = 512


def build_program(nb=NB, tb=TB, n_full=N_FULL, n_cores=NCORES, timing=False,
                  variant="full", lb=3, wb=8, ob=4, pstb=2, psmb=3):
    """Build the SPMD Bass program (same program for all cores).

    timing=True: outputs go to Internal DRAM and a small checksum tensor is
    the only ExternalOutput, so repeated executions can be queued
    back-to-back without re-placing GB-scale donated output buffers.
    """
    dt = mybir.dt
    npad = nb * tb * P
    nc = bacc.Bacc(
        "TRN2",
        target_bir_lowering=False,
        debug=False,
        enable_asserts=True,
        num_devices=n_cores,
    )
    x1d = nc.dram_tensor("x1t", [nb, P, tb * 2, P], dt.bfloat16, kind="ExternalInput").ap()
    a2d = nc.dram_tensor("a2t", [nb, P, tb * 2, P], dt.bfloat16, kind="ExternalInput").ap()
    idxd = nc.dram_tensor("idx", [P, nb * tb * 3], dt.int32, kind="ExternalInput").ap()
    o2fd = nc.dram_tensor("o2f", [n_full, D_IN], dt.bfloat16, kind="ExternalInput").ap()
    wctd = nc.dram_tensor("wct", [4, P, D_OUT], dt.bfloat16, kind="ExternalInput").ap()
    wagd = nc.dram_tensor("wagt", [2, P, D_OUT], dt.bfloat16, kind="ExternalInput").ap()
    bcd = nc.dram_tensor("bc", [P, D_OUT], dt.float32, kind="ExternalInput").ap()
    bad = nc.dram_tensor("ba", [P, D_OUT], dt.float32, kind="ExternalInput").ap()
    bard = nc.dram_tensor("bar", [1, D_OUT], dt.bfloat16, kind="ExternalInput").ap()
    okind = "Internal" if timing else "ExternalOutput"
    o3d = nc.dram_tensor("o3", [npad, D_OUT], dt.bfloat16, kind=okind).ap()
    o4d = nc.dram_tensor("o4", [npad, D_OUT], dt.bfloat16, kind=okind).ap()
    chkd = (nc.dram_tensor("chk", [P, D_OUT], dt.float32, kind="ExternalOutput").ap()
            if timing else None)

    with tile.TileContext(nc) as tc, ExitStack() as ctx:
        const = ctx.enter_context(tc.tile_pool(name="const", bufs=1))
        loadp = ctx.enter_context(tc.tile_pool(name="loads", bufs=lb))
        work = ctx.enter_context(tc.tile_pool(name="work", bufs=wb))
        outp = ctx.enter_context(tc.tile_pool(name="outs", bufs=ob))
        pst = ctx.enter_context(tc.tile_pool(name="pst", bufs=pstb, space="PSUM"))
        psm = ctx.enter_context(tc.tile_pool(name="psm", bufs=psmb, space="PSUM"))

        ident = const.tile([P, P], dt.float32)
        make_identity(nc, ident[:])
        idx_sb = const.tile([P, nb * tb * 3], dt.int32)
        nc.sync.dma_start(idx_sb[:], idxd[:])
        wct_sb = const.tile([P, 4, D_OUT], dt.bfloat16)
        for c in range(4):
            nc.sync.dma_start(wct_sb[:, c, :], wctd[c])
        wag_sb = const.tile([P, 2, D_OUT], dt.bfloat16)
        for c in range(2):
            nc.sync.dma_start(wag_sb[:, c, :], wagd[c])
        bc_sb = const.tile([P, D_OUT], dt.float32)
        nc.sync.dma_start(bc_sb[:], bcd[:])
        ba_sb = const.tile([P, D_OUT], dt.float32)
        nc.sync.dma_start(ba_sb[:], bad[:])

        onesb = const.tile([1, P], dt.bfloat16)
        nc.gpsimd.memset(onesb[:], 1.0)
        barow = const.tile([1, D_OUT], dt.bfloat16)
        nc.sync.dma_start(barow[:], bard[:])

        o3v = o3d.rearrange("(m p) d -> p m d", p=P)
        o4v = o4d.rearrange("(m p) d -> p m d", p=P)

        for b in range(nb):
            x1 = loadp.tile([P, tb * 2, P], dt.bfloat16, tag="x1")
            nc.sync.dma_start(x1[:], x1d[b])
            a2 = loadp.tile([P, tb * 2, P], dt.bfloat16, tag="a2")
            nc.sync.dma_start(a2[:], a2d[b])
            for t in range(tb):
                # 3 neighbour rows gathered with SDMA-side f32 accumulate
                # (HW ucode supports one index per partition per instr)
                gsum = work.tile([P, D_IN], dt.float32, tag="gsum")
                if variant == "nogather":
                    nc.gpsimd.memset(gsum[:], 0.0)
                else:
                    for j in range(3):
                        ki = (b * tb + t) * 3 + j
                        nc.gpsimd.indirect_dma_start(
                            out=gsum[:],
                            out_offset=None,
                            in_=o2fd[:],
                            in_offset=bass.IndirectOffsetOnAxis(
                                ap=idx_sb[:, ki:ki + 1], axis=0
                            ),
                            compute_op=(mybir.AluOpType.bypass if j == 0
                                        else mybir.AluOpType.add),
                        )
                if variant == "gatheronly":
                    continue
                # PE: transpose neighbour-sum to feature-major
                pt = pst.tile([P, 2, P], dt.float32, tag="pt")
                for c in range(2):
                    nc.tensor.matmul(
                        out=pt[:, c, :], lhsT=gsum[:, c * P:(c + 1) * P],
                        rhs=ident[:], is_transpose=True,
                        start=True, stop=True,
                    )
                # evict aggT + add self rows (already feature-major) -> bf16
                aggt = work.tile([P, 2, P], dt.bfloat16, tag="aggt")
                nc.vector.tensor_tensor(
                    out=aggt[:], in0=pt[:], in1=a2[:, t * 2:t * 2 + 2, :],
                    op=mybir.AluOpType.add,
                )
                # comb: concat(out1,out2) @ W_comb.T  (4 K-chunks of 128)
                p3 = psm.tile([P, D_OUT], dt.float32, tag="p3")
                for c in range(4):
                    lhsT = x1[:, t * 2 + c, :] if c < 2 else a2[:, t * 2 + c - 2, :]
                    nc.tensor.matmul(
                        out=p3[:], lhsT=lhsT, rhs=wct_sb[:, c, :],
                        start=(c == 0), stop=(c == 3),
                    )
                m = b * tb + t
                # engine load-balance: 1 in 4 tiles injects out4's bias via a
                # K=1 PE matmul + ACT-copy eviction; the rest use a DVE
                # tensor_tensor eviction (bias add fused)
                pe_bias = (m % 4 == 3)
                p4 = psm.tile([P, D_OUT], dt.float32, tag="p4")
                if pe_bias:
                    nc.tensor.matmul(
                        out=p4[:], lhsT=onesb[:], rhs=barow[:],
                        start=True, stop=False,
                    )
                for c in range(2):
                    nc.tensor.matmul(
                        out=p4[:], lhsT=aggt[:, c, :], rhs=wag_sb[:, c, :],
                        start=(not pe_bias and c == 0), stop=(c == 1),
                    )
                sl = m % 2
                if sl == 0:
                    o3sb = outp.tile([P, 2, D_OUT], dt.bfloat16, tag="o3sb")
                    o4sb = outp.tile([P, 2, D_OUT], dt.bfloat16, tag="o4sb")
                nc.vector.tensor_tensor(
                    out=o3sb[:, sl, :], in0=p3[:], in1=bc_sb[:],
                    op=mybir.AluOpType.add,
                )
                if pe_bias:
                    nc.scalar.copy(out=o4sb[:, sl, :], in_=p4[:])
                else:
                    nc.vector.tensor_tensor(
                        out=o4sb[:, sl, :], in0=p4[:], in1=ba_sb[:],
                        op=mybir.AluOpType.add,
                    )
                if sl == 1 or m == nb * tb - 1:
                    m0 = m - sl
                    nc.sync.dma_start(o3v[:, m0:m + 1, :], o3sb[:, :sl + 1, :])
                    nc.scalar.dma_start(o4v[:, m0:m + 1, :], o4sb[:, :sl + 1, :])

        if timing:
            # checksum depends on the last staged outputs, keeping work live
            chk = outp.tile([P, D_OUT], dt.float32, tag="chk")
            if variant == "gatheronly":
                nc.vector.tensor_copy(chk[:, :D_IN], gsum[:])
                nc.gpsimd.memset(chk[:, D_IN:], 0.0)
            else:
                nc.vector.tensor_tensor(out=chk[:], in0=o3sb[:, 0, :],
                                        in1=o4sb[:, 0, :], op=mybir.AluOpType.add)
            nc.sync.dma_start(chkd[:], chk[:])

    nc.compile()
    return nc


def _pack_T(rows, nb, tb):
    """[rows, 256] f32 -> [nb, P, tb, 2, P] bf16 feature-major tiles."""
    npad = nb * tb * P
    pad = np.zeros((npad, D_IN), BF16)
    pad[: rows.shape[0]] = rows.astype(BF16)
    r = pad.reshape(nb, tb, P, 2, P)               # [b, t, node, c, feat]
    return np.ascontiguousarray(r.transpose(0, 4, 1, 3, 2)).reshape(nb, P, tb * 2, P)  # [b, feat, t*2+c, node]


def _pack_idx(nbr, nb, tb):
    """[rows, 3] int32 -> [P, nb*tb*3] partition-major index layout."""
    npad = nb * tb * P
    pad = np.zeros((npad, 3), np.int32)
    pad[: nbr.shape[0]] = nbr
    r = pad.reshape(nb, tb, P, 3)                  # [b, t, node, j]
    return np.ascontiguousarray(r.transpose(2, 0, 1, 3).reshape(P, nb * tb * 3))


def prep_in_maps(out1, out2, neighbour, W_comb, b_comb, W_agg, b_agg,
                 nb=NB, tb=TB, n_cores=NCORES):
    out1 = np.asarray(out1, dtype=np.float32)
    out2 = np.asarray(out2, dtype=np.float32)
    nbr32 = np.asarray(neighbour).astype(np.int32)
    o2f = np.ascontiguousarray(np.asarray(out2).astype(BF16))
    wct = np.ascontiguousarray(np.asarray(W_comb, dtype=np.float32).T.astype(BF16)).reshape(4, P, D_OUT)
    wag = np.ascontiguousarray((0.25 * np.asarray(W_agg, dtype=np.float32)).T.astype(BF16)).reshape(2, P, D_OUT)
    bc = np.ascontiguousarray(np.tile(np.asarray(b_comb, dtype=np.float32)[None, :], (P, 1)))
    ba = np.ascontiguousarray(np.tile(np.asarray(b_agg, dtype=np.float32)[None, :], (P, 1)))
    bar = np.asarray(b_agg, dtype=np.float32).astype(BF16).reshape(1, D_OUT)
    rpc = out1.shape[0] // n_cores
    in_maps = []
    for i in range(n_cores):
        sl = slice(i * rpc, (i + 1) * rpc)
        in_maps.append(dict(
            x1t=_pack_T(out1[sl], nb, tb),
            a2t=_pack_T(out2[sl], nb, tb),
            idx=_pack_idx(nbr32[sl], nb, tb),
            o2f=o2f, wct=wct, wagt=wag, bc=bc, ba=ba, bar=bar,
        ))
    return in_maps


_NC_CACHE = {}


def _get_program():
    key = (NB, TB, N_FULL)
    if key not in _NC_CACHE:
        _NC_CACHE[key] = build_program()
    return _NC_CACHE[key]


def kernel(out1, out2, neighbour, W_comb, b_comb, W_agg, b_agg, _trace=False, **kw):
    nc = _get_program()
    in_maps = prep_in_maps(out1, out2, neighbour, W_comb, b_comb, W_agg, b_agg)
    res = run_bass_kernel_spmd(nc, in_maps, list(range(NCORES)), trace=_trace, **kw)
    out3 = np.concatenate([res.results[i]["o3"][:RPC].astype(np.float32) for i in range(NCORES)], axis=0)
    out4 = np.concatenate([res.results[i]["o4"][:RPC].astype(np.float32) for i in range(NCORES)], axis=0)
    if _trace:
        return (out3, out4), res
    return (out3, out4)
